# revision 18
# baseline (speedup 1.0000x reference)
"""Trainium2 Bass kernel for the NRI encoder problem.

Math: the reference's construct_pair makes pair[n,i,j,:] = concat(h[n,i], h[n,i])
(independent of the receiver axis j), so the (m,m) edge grid collapses:
  edge[n,i,j,:]   = E[n,i,:]            where E = f_edge(concat(h,h))
  e2n[n,j,:]      = sum_{i!=j} E[n,i,:] = S[n] - E[n,j,:],  S[n] = sum_i E[n,i]
  h2              = f_e2n(e2n)
  edge2[n,i,j]    = f_n2e(concat(h2[n,i],h2[n,i])) / m      (broadcast over j)

Sharding: 8 cores, each handles 128 of the 1024 (batch, node) rows. Every core
redundantly computes h/E/S for its whole batch element (tiny: m=512, L=128),
then runs the last two FFNs and the output writes only for its own 128 nodes.
The per-core input x slice is rolled so the core's nodes sit at positions
0..127; the global sum S is permutation invariant, so results are exact.

Layout is feature-major on chip (features on partitions, nodes on the free
axis) so no activation ever needs transposing; weights (din,dout) are already
in matmul lhsT layout. concat(h,h) @ W is folded host-side to h @ (W_top+W_bot).

Perf notes (from perfetto traces): matmuls run as float32r (single PE pass vs
fp32's two half-rate passes); inputs arrive in three merged DMAs (each DIRECT2D
issue costs ~0.6-1us on a sequencer) split over the two HWDGE rings; the
512-node stages are split into two 256-column chunks so PE/ACT/DVE pipeline
instead of serializing; output DMAs are split across both rings.
"""

import numpy as np

L = 128
M = 512
N_B = 2
D_IN = 6
N_CORES = 8
RPC = 128  # rows (nodes) per core
CH = 256   # column chunk for the 512-node stages

_CACHE = {}


def _ensure_path():
    try:
        import concourse  # noqa: F401
    except ImportError:
        import sys
        for p in ("/opt/trn_rl_repo", "/root/.axon_site/_ro/trn_rl_repo"):
            if p not in sys.path:
                sys.path.insert(0, p)


def _build_bass():
    _ensure_path()
    import concourse.mybir as mybir
    from concourse import bacc
    from concourse.tile import TileContext
    from concourse.masks import make_identity

    f32 = mybir.dt.float32
    f32r = mybir.dt.float32r
    AF = mybir.ActivationFunctionType
    OP = mybir.AluOpType

    nc = bacc.Bacc()
    # gaw: 10 bias cols | Wn2 (2x128).  gbcd: We1f|We2 | Wen1|Wen2 | Wne1f|Wne2
    gaw_d = nc.declare_dram_parameter("gaw", [128, 266], f32r, isOutput=False)
    gbcd_d = nc.declare_dram_parameter("gbcd", [128, 898], f32r, isOutput=False)
    small_d = nc.declare_dram_parameter("small", [D_IN, 256 + M], f32r, isOutput=False)
    h2o_d = nc.declare_dram_parameter("h2o", [RPC, L], f32, isOutput=True)
    edo_d = nc.declare_dram_parameter("edo", [RPC, M], f32, isOutput=True)

    with TileContext(nc) as tc:
        with (
            tc.tile_pool(name="w", bufs=1) as wp,
            tc.tile_pool(name="act", bufs=1) as sp,
            tc.tile_pool(name="psA", bufs=5, space="PSUM") as ppa,
            tc.tile_pool(name="psB", bufs=3, space="PSUM") as ppb,
        ):
            small = wp.tile([D_IN, 256 + M], f32r)
            gaw = wp.tile([128, 266], f32r)
            gbcd = wp.tile([128, 898], f32r)
            ident = wp.tile([128, 128], f32)

            nc.sync.dma_start(out=small[:], in_=small_d[:])
            nc.scalar.dma_start(out=gaw[:], in_=gaw_d[:])
            nc.scalar.dma_start(out=gbcd[:], in_=gbcd_d[:])
            make_identity(nc, ident[:])

            # PE warm-up: the HAM throttle only reaches full clock after ~4us
            # of continuous PE work; burn the dead window while the input DMAs
            # are in flight so the real matmuls run at the warm rate.
            ps_w = ppb.tile([128, 128], f32, tag="psB")
            for _ in range(5):
                nc.tensor.matmul(ps_w[:], lhsT=ident[:], rhs=ident[:])

            b = lambda i: gaw[:, i:i + 1].bitcast(f32)  # noqa: E731  bias col
            w2 = gaw[:, 10:266]     # Wn2 halves
            gb = gbcd[:, 0:256]     # We1f | We2
            gc = gbcd[:, 256:768]   # Wen1 a|b, Wen2 a|b
            gd = gbcd[:, 768:898]   # Wne1f | Wne2 | 0-pad

            # ACT instructions encode one sync wait; observe the gaw DMA here
            # so later activations only wait on PE.
            warm = sp.tile([128, 1], f32)
            nc.scalar.copy(warm[:], b(0))

            # ---- node/edge stages over 512 nodes, 2 chunks of 256 columns.
            # Emission is STAGE-major: engines execute in program order, so
            # interleaving chunks per stage is what lets chunk 1's matmuls run
            # while chunk 0's bias/relu ops are still on ACT/DVE.
            et = sp.tile([128, M], f32)
            xc = [small[0:D_IN, 256 + c * CH:256 + (c + 1) * CH] for c in range(2)]

            # f_node layer 1: H1 = relu(Wn1^T x + bn1), halves on ACT / DVE
            ps_h1a, ps_h1b, h1a, h1b = [], [], [], []
            for c in range(2):
                ps_h1a.append(ppa.tile([128, CH], f32, tag="psA", name=f"ph1a{c}"))
                nc.tensor.matmul(ps_h1a[c][:], lhsT=small[0:D_IN, 0:128], rhs=xc[c])
                ps_h1b.append(ppa.tile([128, CH], f32, tag="psA", name=f"ph1b{c}"))
                nc.tensor.matmul(ps_h1b[c][:], lhsT=small[0:D_IN, 128:256], rhs=xc[c])
            for c in range(2):
                h1a.append(sp.tile([128, CH], f32r, tag=f"h1a{c}", name=f"h1a{c}"))
                nc.scalar.activation(h1a[c][:], ps_h1a[c][:], AF.Relu,
                                     bias=b(0), scale=1.0)
                h1b.append(sp.tile([128, CH], f32r, tag=f"h1b{c}", name=f"h1b{c}"))
                nc.vector.tensor_scalar(
                    out=h1b[c][:], in0=ps_h1b[c][:], scalar1=b(1), scalar2=0.0,
                    op0=OP.add, op1=OP.max,
                )

            # f_node layer 2: Ht = Wn2^T H1 + bn2 (ht_0 on ACT, ht_1 on DVE)
            ps_h, ht = [], []
            for c in range(2):
                ps_h.append(ppa.tile([128, CH], f32, tag="psA", name=f"ph{c}"))
                nc.tensor.matmul(ps_h[c][:], lhsT=w2[:, 0:128], rhs=h1a[c][:],
                                 start=True, stop=False)
                nc.tensor.matmul(ps_h[c][:], lhsT=w2[:, 128:256], rhs=h1b[c][:],
                                 start=False, stop=True)
            for c in range(2):
                ht.append(sp.tile([128, CH], f32r, tag=f"ht{c}", name=f"ht{c}"))
                if c == 0:
                    nc.scalar.activation(ht[c][:], ps_h[c][:], AF.Identity,
                                         bias=b(2), scale=1.0)
                else:
                    nc.vector.tensor_scalar_add(ht[c][:], ps_h[c][:], b(2))

            # f_edge layer 1: A1 = relu(We1f^T Ht + be1)
            ps_a1, a1 = [], []
            for c in range(2):
                ps_a1.append(ppa.tile([128, CH], f32, tag="psA", name=f"pa1{c}"))
                nc.tensor.matmul(ps_a1[c][:], lhsT=gb[:, 0:128], rhs=ht[c][:])
            for c in range(2):
                a1.append(sp.tile([128, CH], f32r, tag=f"a1{c}", name=f"a1{c}"))
                nc.scalar.activation(a1[c][:], ps_a1[c][:], AF.Relu,
                                     bias=b(3), scale=1.0)

            # f_edge layer 2: Et = We2^T A1 + be2, fused per-chunk row-sum
            ps_e, ssum_c = [], []
            for c in range(2):
                ps_e.append(ppa.tile([128, CH], f32, tag="psA", name=f"pe{c}"))
                nc.tensor.matmul(ps_e[c][:], lhsT=gb[:, 128:256], rhs=a1[c][:])
            for c in range(2):
                ssum_c.append(sp.tile([128, 1], f32, tag=f"ss{c}", name=f"ssum{c}"))
                # chunk 1 initializes its accumulator from chunk 0's partial
                # sum (accum = op1(reduce(out), scalar2)), fusing the cross-
                # chunk add into the same instruction.
                nc.vector.tensor_scalar(
                    out=et[:, c * CH:(c + 1) * CH], in0=ps_e[c][:], scalar1=b(4),
                    scalar2=None if c == 0 else ssum_c[0][:],
                    op0=OP.add, op1=OP.add, accum_out=ssum_c[c][:],
                )
            ssum = ssum_c[1]

            # e2n = S - E, own nodes only (first RPC columns)
            e2n = sp.tile([128, RPC], f32r)
            nc.vector.tensor_sub(
                e2n[:], ssum[:].broadcast_to([128, RPC]), et[:, 0:RPC]
            )

            # f_e2n layer 1: A2 = relu(Wen1^T e2n + ben1), halves on ACT / DVE
            ps_a2a = ppb.tile([128, RPC], f32, tag="psB")
            nc.tensor.matmul(ps_a2a[:], lhsT=gc[:, 0:128], rhs=e2n[:])
            a2a = sp.tile([128, RPC], f32r)
            nc.scalar.activation(a2a[:], ps_a2a[:], AF.Relu, bias=b(5), scale=1.0)

            ps_a2b = ppb.tile([128, RPC], f32, tag="psB")
            nc.tensor.matmul(ps_a2b[:], lhsT=gc[:, 128:256], rhs=e2n[:])
            a2b = sp.tile([128, RPC], f32r)
            nc.vector.tensor_scalar(
                out=a2b[:], in0=ps_a2b[:], scalar1=b(6), scalar2=0.0,
                op0=OP.add, op1=OP.max,
            )

            # f_e2n layer 2: H2t = Wen2^T A2 + ben2
            ps_h2 = ppb.tile([128, RPC], f32, tag="psB")
            nc.tensor.matmul(ps_h2[:], lhsT=gc[:, 256:384], rhs=a2a[:],
                             start=True, stop=False)
            nc.tensor.matmul(ps_h2[:], lhsT=gc[:, 384:512], rhs=a2b[:],
                             start=False, stop=True)
            h2t = sp.tile([128, RPC], f32r)
            nc.scalar.activation(h2t[:], ps_h2[:], AF.Identity, bias=b(7), scale=1.0)

            # h2 transpose (PE) — the h2n copy is emitted AFTER the a3 relu so
            # ACT doesn't delay the edge-output critical path with it
            ps_h2n = ppb.tile([RPC, L], f32, tag="psB")
            nc.tensor.transpose(ps_h2n[:], h2t[:].bitcast(f32), ident[:])

            # f_node2edge layer 1: A3 = relu(Wne1f^T H2t + bne1)
            ps_a3 = ppb.tile([128, RPC], f32, tag="psB")
            nc.tensor.matmul(ps_a3[:], lhsT=gd[:, 0:128], rhs=h2t[:])
            a3 = sp.tile([128, RPC], f32r)
            nc.scalar.activation(a3[:], ps_a3[:], AF.Relu, bias=b(8), scale=1.0)

            # f_node2edge layer 2 (per own node): s = (A3^T Wne2)/m + bne2/m
            ps_s = ppb.tile([RPC, 2], f32, tag="psB")
            nc.tensor.matmul(ps_s[:], lhsT=a3[:], rhs=gd[:, 128:130])
            scol = sp.tile([RPC, 1], f32)
            nc.vector.tensor_scalar(
                out=scol[:], in0=ps_s[:, 0:1], scalar1=1.0 / M, scalar2=b(9),
                op0=OP.mult, op1=OP.add,
            )

            # edge2 rows: own node's scalar broadcast across all m columns,
            # half at a time so the first DMA issues while the second half is
            # still being written; halves go to separate queues
            edt = sp.tile([RPC, M], f32)
            nc.vector.tensor_copy(out=edt[:, 0:256],
                                  in_=scol[:].broadcast_to([RPC, 256]))
            nc.sync.dma_start(out=edo_d[:, 0:256], in_=edt[:, 0:256])
            nc.vector.tensor_copy(out=edt[:, 256:512],
                                  in_=scol[:].broadcast_to([RPC, 256]))
            nc.scalar.dma_start(out=edo_d[:, 256:512], in_=edt[:, 256:512])

            # h2 slice out (off the critical path)
            h2n = sp.tile([RPC, L], f32)
            nc.scalar.copy(h2n[:], ps_h2n[:])
            nc.sync.dma_start(out=h2o_d[:], in_=h2n[:])

    if not nc.is_finalized():
        nc.finalize()
    return nc


def _prep_gaw(Wn2, bn1, bn2, be1, be2, ben1, ben2, bne1, bne2):
    gaw = np.zeros((128, 266), np.float32)
    gaw[:, 0] = bn1[:128]
    gaw[:, 1] = bn1[128:]
    gaw[:, 2] = bn2
    gaw[:, 3] = be1
    gaw[:, 4] = be2
    gaw[:, 5] = ben1[:128]
    gaw[:, 6] = ben1[128:]
    gaw[:, 7] = ben2
    gaw[:, 8] = bne1
    gaw[:, 9] = bne2[0] / np.float32(M)
    gaw[:, 10:138] = Wn2[:128]
    gaw[:, 138:266] = Wn2[128:]
    return gaw


def _prep_gbcd(We1, We2, Wen1, Wen2, Wne1, Wne2):
    gbcd = np.zeros((128, 898), np.float32)
    gbcd[:, 0:128] = We1[:128] + We1[128:]
    gbcd[:, 128:256] = We2
    gbcd[:, 256:384] = Wen1[:, :128]
    gbcd[:, 384:512] = Wen1[:, 128:]
    gbcd[:, 512:640] = Wen2[:128]
    gbcd[:, 640:768] = Wen2[128:]
    gbcd[:, 768:896] = Wne1[:128] + Wne1[128:]
    gbcd[:, 896] = Wne2[:, 0]
    return gbcd


def kernel(x, Wn1, bn1, Wn2, bn2, We1, be1, We2, be2,
           Wen1, ben1, Wen2, ben2, Wne1, bne1, Wne2, bne2):
    _ensure_path()
    from concourse.bass_utils import run_bass_kernel_spmd

    f = np.float32
    x = np.asarray(x, f)
    gaw = _prep_gaw(*[np.asarray(a, f) for a in
                      (Wn2, bn1, bn2, be1, be2, ben1, ben2, bne1, bne2)])
    gbcd = _prep_gbcd(*[np.asarray(a, f) for a in
                        (We1, We2, Wen1, Wen2, Wne1, Wne2)])
    Wn1 = np.asarray(Wn1, f)

    in_maps = []
    for k in range(N_CORES):
        n, r = divmod(k, N_CORES // N_B)
        small = np.empty((D_IN, 256 + M), f)
        small[:, 0:256] = Wn1
        small[:, 256:] = np.roll(x[n], -r * RPC, axis=0).T
        in_maps.append({"gaw": gaw, "gbcd": gbcd, "small": small})

    if "nc" not in _CACHE:
        _CACHE["nc"] = _build_bass()
    res = run_bass_kernel_spmd(_CACHE["nc"], in_maps, list(range(N_CORES))).results

    h2 = np.empty((N_B, M, L), f)
    edge2 = np.empty((N_B, M, M), f)
    for k in range(N_CORES):
        n, r = divmod(k, N_CORES // N_B)
        h2[n, r * RPC:(r + 1) * RPC] = res[k]["h2o"]
        edge2[n, r * RPC:(r + 1) * RPC] = res[k]["edo"]
    return h2, edge2


# revision 19
# speedup vs baseline: 1.1893x; 1.1893x over previous
"""Trainium2 Bass kernel for the NRI encoder problem.

Math: the reference's construct_pair makes pair[n,i,j,:] = concat(h[n,i], h[n,i])
(independent of the receiver axis j), so the (m,m) edge grid collapses:
  edge[n,i,j,:]   = E[n,i,:]            where E = f_edge(concat(h,h))
  e2n[n,j,:]      = sum_{i!=j} E[n,i,:] = S[n] - E[n,j,:],  S[n] = sum_i E[n,i]
  h2              = f_e2n(e2n)
  edge2[n,i,j]    = f_n2e(concat(h2[n,i],h2[n,i])) / m      (broadcast over j)

Sharding: 8 cores, each handles 128 of the 1024 (batch, node) rows. Every core
redundantly computes h/E/S for its whole batch element (tiny: m=512, L=128),
then runs the last two FFNs and the output writes only for its own 128 nodes.
The per-core input x slice is rolled so the core's nodes sit at positions
0..127; the global sum S is permutation invariant, so results are exact.

Layout is feature-major on chip (features on partitions, nodes on the free
axis) so no activation ever needs transposing; weights (din,dout) are already
in matmul lhsT layout. concat(h,h) @ W is folded host-side to h @ (W_top+W_bot).

Perf notes (from perfetto traces): matmuls run as float32r (single PE pass vs
fp32's two half-rate passes); inputs arrive in three merged DMAs (each DIRECT2D
issue costs ~0.6-1us on a sequencer) split over the two HWDGE rings; the
512-node stages are split into two 256-column chunks so PE/ACT/DVE pipeline
instead of serializing; output DMAs are split across both rings.
"""

import numpy as np

L = 128
M = 512
N_B = 2
D_IN = 6
N_CORES = 8
RPC = 128  # rows (nodes) per core
CH = 256   # column chunk for the 512-node stages

_CACHE = {}


def _ensure_path():
    try:
        import concourse  # noqa: F401
    except ImportError:
        import sys
        for p in ("/opt/trn_rl_repo", "/root/.axon_site/_ro/trn_rl_repo"):
            if p not in sys.path:
                sys.path.insert(0, p)


def _build_bass():
    _ensure_path()
    import concourse.mybir as mybir
    from concourse import bacc
    from concourse.tile import TileContext
    from concourse.masks import make_identity

    f32 = mybir.dt.float32
    f32r = mybir.dt.float32r
    AF = mybir.ActivationFunctionType
    OP = mybir.AluOpType

    nc = bacc.Bacc()
    # gaw: 10 bias cols | Wn2 (2x128).  gbcd: We1f|We2 | Wen1|Wen2 | Wne1f|Wne2
    gaw_d = nc.declare_dram_parameter("gaw", [128, 266], f32r, isOutput=False)
    gbcd_d = nc.declare_dram_parameter("gbcd", [128, 898], f32r, isOutput=False)
    small_d = nc.declare_dram_parameter("small", [D_IN, 256 + M], f32r, isOutput=False)
    h2o_d = nc.declare_dram_parameter("h2o", [RPC, L], f32, isOutput=True)
    edo_d = nc.declare_dram_parameter("edo", [RPC, M], f32, isOutput=True)

    with TileContext(nc) as tc:
        with (
            tc.tile_pool(name="w", bufs=1) as wp,
            tc.tile_pool(name="act", bufs=1) as sp,
            tc.tile_pool(name="psA", bufs=5, space="PSUM") as ppa,
            tc.tile_pool(name="psB", bufs=3, space="PSUM") as ppb,
        ):
            small = wp.tile([D_IN, 256 + M], f32r)
            gaw = wp.tile([128, 266], f32r)
            gbcd = wp.tile([128, 898], f32r)
            ident = wp.tile([128, 128], f32)

            nc.sync.dma_start(out=small[:], in_=small_d[:])
            nc.scalar.dma_start(out=gaw[:], in_=gaw_d[:])
            nc.scalar.dma_start(out=gbcd[:], in_=gbcd_d[:])
            make_identity(nc, ident[:])

            # PE warm-up: the HAM throttle only reaches full clock after ~4us
            # of continuous PE work; burn the dead window while the input DMAs
            # are in flight so the real matmuls run at the warm rate.
            ps_w = ppb.tile([128, 128], f32, tag="psB")
            for _ in range(5):
                nc.tensor.matmul(ps_w[:], lhsT=ident[:], rhs=ident[:])

            b = lambda i: gaw[:, i:i + 1].bitcast(f32)  # noqa: E731  bias col
            w2 = gaw[:, 10:266]     # Wn2 halves
            gb = gbcd[:, 0:256]     # We1f | We2
            gc = gbcd[:, 256:768]   # Wen1 a|b, Wen2 a|b
            gd = gbcd[:, 768:898]   # Wne1f | Wne2 | 0-pad

            # ACT instructions encode one sync wait; observe the gaw DMA here
            # so later activations only wait on PE.
            warm = sp.tile([128, 1], f32)
            nc.scalar.copy(warm[:], b(0))

            # ---- node/edge stages over 512 nodes, 2 chunks of 256 columns.
            # Emission is STAGE-major: engines execute in program order, so
            # interleaving chunks per stage is what lets chunk 1's matmuls run
            # while chunk 0's bias/relu ops are still on ACT/DVE.
            et = sp.tile([128, M], f32)
            xc = [small[0:D_IN, 256 + c * CH:256 + (c + 1) * CH] for c in range(2)]

            # f_node layer 1: H1 = relu(Wn1^T x + bn1), halves on ACT / DVE
            ps_h1a, ps_h1b, h1a, h1b = [], [], [], []
            for c in range(2):
                ps_h1a.append(ppa.tile([128, CH], f32, tag="psA", name=f"ph1a{c}"))
                nc.tensor.matmul(ps_h1a[c][:], lhsT=small[0:D_IN, 0:128], rhs=xc[c])
                ps_h1b.append(ppa.tile([128, CH], f32, tag="psA", name=f"ph1b{c}"))
                nc.tensor.matmul(ps_h1b[c][:], lhsT=small[0:D_IN, 128:256], rhs=xc[c])
            for c in range(2):
                h1a.append(sp.tile([128, CH], f32r, tag=f"h1a{c}", name=f"h1a{c}"))
                nc.scalar.activation(h1a[c][:], ps_h1a[c][:], AF.Relu,
                                     bias=b(0), scale=1.0)
                h1b.append(sp.tile([128, CH], f32r, tag=f"h1b{c}", name=f"h1b{c}"))
                nc.vector.tensor_scalar(
                    out=h1b[c][:], in0=ps_h1b[c][:], scalar1=b(1), scalar2=0.0,
                    op0=OP.add, op1=OP.max,
                )

            # f_node layer 2: Ht = Wn2^T H1 + bn2 (ht_0 on ACT, ht_1 on DVE)
            ps_h, ht = [], []
            for c in range(2):
                ps_h.append(ppa.tile([128, CH], f32, tag="psA", name=f"ph{c}"))
                nc.tensor.matmul(ps_h[c][:], lhsT=w2[:, 0:128], rhs=h1a[c][:],
                                 start=True, stop=False)
                nc.tensor.matmul(ps_h[c][:], lhsT=w2[:, 128:256], rhs=h1b[c][:],
                                 start=False, stop=True)
            for c in range(2):
                ht.append(sp.tile([128, CH], f32r, tag=f"ht{c}", name=f"ht{c}"))
                if c == 0:
                    nc.scalar.activation(ht[c][:], ps_h[c][:], AF.Identity,
                                         bias=b(2), scale=1.0)
                else:
                    nc.vector.tensor_scalar_add(ht[c][:], ps_h[c][:], b(2))

            # f_edge layer 1: A1 = relu(We1f^T Ht + be1)
            ps_a1, a1 = [], []
            for c in range(2):
                ps_a1.append(ppa.tile([128, CH], f32, tag="psA", name=f"pa1{c}"))
                nc.tensor.matmul(ps_a1[c][:], lhsT=gb[:, 0:128], rhs=ht[c][:])
            for c in range(2):
                a1.append(sp.tile([128, CH], f32r, tag=f"a1{c}", name=f"a1{c}"))
                nc.scalar.activation(a1[c][:], ps_a1[c][:], AF.Relu,
                                     bias=b(3), scale=1.0)

            # f_edge layer 2: Et = We2^T A1 + be2, fused per-chunk row-sum
            ps_e, ssum_c = [], []
            for c in range(2):
                ps_e.append(ppa.tile([128, CH], f32, tag="psA", name=f"pe{c}"))
                nc.tensor.matmul(ps_e[c][:], lhsT=gb[:, 128:256], rhs=a1[c][:])
            for c in range(2):
                ssum_c.append(sp.tile([128, 1], f32, tag=f"ss{c}", name=f"ssum{c}"))
                # chunk 1 initializes its accumulator from chunk 0's partial
                # sum (accum = op1(reduce(out), scalar2)), fusing the cross-
                # chunk add into the same instruction.
                nc.vector.tensor_scalar(
                    out=et[:, c * CH:(c + 1) * CH], in0=ps_e[c][:], scalar1=b(4),
                    scalar2=None if c == 0 else ssum_c[0][:],
                    op0=OP.add, op1=OP.add, accum_out=ssum_c[c][:],
                )
            ssum = ssum_c[1]

            # e2n = S - E, own nodes only (first RPC columns)
            e2n = sp.tile([128, RPC], f32r)
            nc.vector.tensor_sub(
                e2n[:], ssum[:].broadcast_to([128, RPC]), et[:, 0:RPC]
            )

            # f_e2n layer 1: A2 = relu(Wen1^T e2n + ben1), halves on ACT / DVE
            ps_a2a = ppb.tile([128, RPC], f32, tag="psB")
            nc.tensor.matmul(ps_a2a[:], lhsT=gc[:, 0:128], rhs=e2n[:])
            a2a = sp.tile([128, RPC], f32r)
            nc.scalar.activation(a2a[:], ps_a2a[:], AF.Relu, bias=b(5), scale=1.0)

            ps_a2b = ppb.tile([128, RPC], f32, tag="psB")
            nc.tensor.matmul(ps_a2b[:], lhsT=gc[:, 128:256], rhs=e2n[:])
            a2b = sp.tile([128, RPC], f32r)
            nc.vector.tensor_scalar(
                out=a2b[:], in0=ps_a2b[:], scalar1=b(6), scalar2=0.0,
                op0=OP.add, op1=OP.max,
            )

            # f_e2n layer 2: H2t = Wen2^T A2 + ben2
            ps_h2 = ppb.tile([128, RPC], f32, tag="psB")
            nc.tensor.matmul(ps_h2[:], lhsT=gc[:, 256:384], rhs=a2a[:],
                             start=True, stop=False)
            nc.tensor.matmul(ps_h2[:], lhsT=gc[:, 384:512], rhs=a2b[:],
                             start=False, stop=True)
            h2t = sp.tile([128, RPC], f32r)
            nc.scalar.activation(h2t[:], ps_h2[:], AF.Identity, bias=b(7), scale=1.0)

            # h2 slice out: transpose (features, nodes) -> (nodes, features)
            ps_h2n = ppb.tile([RPC, L], f32, tag="psB")
            nc.tensor.transpose(ps_h2n[:], h2t[:].bitcast(f32), ident[:])

            # f_node2edge layer 1: A3 = relu(Wne1f^T H2t + bne1).  Emitted
            # before the h2n copy so ACT doesn't delay the edge critical path.
            ps_a3 = ppb.tile([128, RPC], f32, tag="psB")
            nc.tensor.matmul(ps_a3[:], lhsT=gd[:, 0:128], rhs=h2t[:])
            a3 = sp.tile([128, RPC], f32r)
            nc.scalar.activation(a3[:], ps_a3[:], AF.Relu, bias=b(8), scale=1.0)

            h2n = sp.tile([RPC, L], f32)
            nc.scalar.copy(h2n[:], ps_h2n[:])
            nc.sync.dma_start(out=h2o_d[:], in_=h2n[:])

            # f_node2edge layer 2 (per own node): s = (A3^T Wne2)/m + bne2/m
            ps_s = ppb.tile([RPC, 2], f32, tag="psB")
            nc.tensor.matmul(ps_s[:], lhsT=a3[:], rhs=gd[:, 128:130])
            scol = sp.tile([RPC, 1], f32)
            nc.vector.tensor_scalar(
                out=scol[:], in0=ps_s[:, 0:1], scalar1=1.0 / M, scalar2=b(9),
                op0=OP.mult, op1=OP.add,
            )

            # edge2 rows: own node's scalar broadcast across all m columns,
            # half at a time so the first DMA issues while the second half is
            # still being written; halves go to separate queues
            edt = sp.tile([RPC, M], f32)
            nc.vector.tensor_copy(out=edt[:, 0:256],
                                  in_=scol[:].broadcast_to([RPC, 256]))
            nc.sync.dma_start(out=edo_d[:, 0:256], in_=edt[:, 0:256])
            nc.vector.tensor_copy(out=edt[:, 256:512],
                                  in_=scol[:].broadcast_to([RPC, 256]))
            nc.scalar.dma_start(out=edo_d[:, 256:512], in_=edt[:, 256:512])

    if not nc.is_finalized():
        nc.finalize()
    return nc


def _prep_gaw(Wn2, bn1, bn2, be1, be2, ben1, ben2, bne1, bne2):
    gaw = np.zeros((128, 266), np.float32)
    gaw[:, 0] = bn1[:128]
    gaw[:, 1] = bn1[128:]
    gaw[:, 2] = bn2
    gaw[:, 3] = be1
    gaw[:, 4] = be2
    gaw[:, 5] = ben1[:128]
    gaw[:, 6] = ben1[128:]
    gaw[:, 7] = ben2
    gaw[:, 8] = bne1
    gaw[:, 9] = bne2[0] / np.float32(M)
    gaw[:, 10:138] = Wn2[:128]
    gaw[:, 138:266] = Wn2[128:]
    return gaw


def _prep_gbcd(We1, We2, Wen1, Wen2, Wne1, Wne2):
    gbcd = np.zeros((128, 898), np.float32)
    gbcd[:, 0:128] = We1[:128] + We1[128:]
    gbcd[:, 128:256] = We2
    gbcd[:, 256:384] = Wen1[:, :128]
    gbcd[:, 384:512] = Wen1[:, 128:]
    gbcd[:, 512:640] = Wen2[:128]
    gbcd[:, 640:768] = Wen2[128:]
    gbcd[:, 768:896] = Wne1[:128] + Wne1[128:]
    gbcd[:, 896] = Wne2[:, 0]
    return gbcd


def kernel(x, Wn1, bn1, Wn2, bn2, We1, be1, We2, be2,
           Wen1, ben1, Wen2, ben2, Wne1, bne1, Wne2, bne2):
    _ensure_path()
    from concourse.bass_utils import run_bass_kernel_spmd

    f = np.float32
    x = np.asarray(x, f)
    gaw = _prep_gaw(*[np.asarray(a, f) for a in
                      (Wn2, bn1, bn2, be1, be2, ben1, ben2, bne1, bne2)])
    gbcd = _prep_gbcd(*[np.asarray(a, f) for a in
                        (We1, We2, Wen1, Wen2, Wne1, Wne2)])
    Wn1 = np.asarray(Wn1, f)

    in_maps = []
    for k in range(N_CORES):
        n, r = divmod(k, N_CORES // N_B)
        small = np.empty((D_IN, 256 + M), f)
        small[:, 0:256] = Wn1
        small[:, 256:] = np.roll(x[n], -r * RPC, axis=0).T
        in_maps.append({"gaw": gaw, "gbcd": gbcd, "small": small})

    if "nc" not in _CACHE:
        _CACHE["nc"] = _build_bass()
    res = run_bass_kernel_spmd(_CACHE["nc"], in_maps, list(range(N_CORES))).results

    h2 = np.empty((N_B, M, L), f)
    edge2 = np.empty((N_B, M, M), f)
    for k in range(N_CORES):
        n, r = divmod(k, N_CORES // N_B)
        h2[n, r * RPC:(r + 1) * RPC] = res[k]["h2o"]
        edge2[n, r * RPC:(r + 1) * RPC] = res[k]["edo"]
    return h2, edge2
